# revision 38
# baseline (speedup 1.0000x reference)
"""Linear-attention (elu feature map) Bass kernel for Trainium2, 8 NeuronCores.

Problem: B=4, H=8, S=8192, D=64 fp32.
  qe = elu(q)+1, ke = elu(k)+1, masked by q_mask/kv_mask
  KV = ke^T @ ve (contract S), ksum = sum_s ke*km
  out = (qe @ KV) / (qe . ksum + 1e-6) * q_mask

Sharding: (B,H) = 32 pairs over 8 cores -> 4 pairs/core, one b per core.

v2 design (host does layout only; all FLOPs on device):
  * Row compaction: kv_mask/q_mask zero ~half the rows and every
    contraction is order-invariant over S, so the host gathers only the
    unmasked rows (padded to a common 128-multiple across cores for
    SPMD). Masked k rows never reach the device; masked q rows are
    zero-filled on the host during scatter-back. This halves DMA and
    every engine's work and removes all on-device mask multiplies.
  * k is shipped compacted; padding rows get -300 so ke = elu+1 = 0.
  * v is shipped as [v | 1] (65 wide); the ones column yields ksum in
    the same accumulated matmul that makes KV.
  * q is shipped pre-transposed and duo-packed: q_t[duo] = [128, NQ]
    bf16, partitions 0-63 = even pair's 64 dims, 64-127 = odd pair's.
    MM2 weights slice straight out of this tile - no PE transposes.
  * elu+1 == min(exp(x), relu(x)+1) exactly: exp on ACT,
    relu+1 (dual-op tensor_scalar, 4x) and min (2x) on DVE, all bf16.
  * MM1 per pair: 128-row chunks accumulate ke^T @ [v|1]; the even
    pair lands in PSUM partitions 0-63 (array col group 0), the odd
    pair in 64-127 (col group 64, via out.base_partition), so one
    [128, 65] bank holds the duo's stacked KV for MM2's two halves.
  * MM2 per 128-col chunk of q_t: out[128s, 65] = qe_chunk^T.T @ kv128
    half; groups of 7 same-parity chunks share a PSUM bank; epilogue
    per bank: rec = recip(den cols), out = num * rec (broadcast AP) on
    DVE, except the small-slab groups which detour through an ACT
    PSUM->SBUF copy + GpSimd multiply to offload the DVE.
  * Software pipeline at duo granularity: A(d) = k/v slabs + MM1;
    B(d) = q prep + MM2 + epilogue + out DMA; B(d) overlaps A(d+1).
    Slabs are (4, rest): a small first slab starts the MM stream early,
    the big slab amortizes DVE/ACT fixed costs and semaphore traffic.
"""
import os
import sys

sys.path.insert(0, "/opt/trn_rl_repo")

import numpy as np
import ml_dtypes

import concourse.bass as bass
import concourse.tile as tile
from concourse import mybir
import bass_rust
from concourse.bass_utils import run_bass_kernel_spmd

B, H, S, D = 4, 8, 8192, 64
PAIRS = 4
DUOS = 2
CPS = 16  # chunks per slab (slab = CPS*128 rows)
F32 = mybir.dt.float32
BF16 = mybir.dt.bfloat16

LAST_RESULT = None


def _split_multi_waits(nc, max_waits=1):
    """walrus setupSyncWait rejects >1 sem wait on one instruction; hoist
    extras onto preceding NoOps on the same engine."""
    for fn in nc.m.functions:
        for bb in fn.blocks:
            insts = list(bb.instructions)
            out = []
            changed = False
            for inst in insts:
                si = getattr(inst, "sync_info", None)
                ow = list(si.on_wait) if si is not None and si.on_wait else []
                if len(ow) > max_waits:
                    changed = True
                    for j, w in enumerate(ow[:-max_waits]):
                        nop = mybir.InstNoOp(
                            name=f"{inst.name}-splitw{j}", ins=[], outs=[]
                        )
                        nop.engine = inst.engine
                        nop.sync_info = bass_rust.SyncInfo(on_wait=[w], on_update=[])
                        out.append(nop)
                    inst.sync_info = bass_rust.SyncInfo(
                        on_wait=ow[-max_waits:], on_update=list(si.on_update or [])
                    )
                out.append(inst)
            if changed:
                bb.instructions = out


def _bcast_inner(ap, n):
    """Append a step-0 inner dim reading each element n times."""
    ap = ap[:, :]
    ap.ap.append([0, n])
    return ap


def _slabs(nchunks):
    """Split nchunks into slabs of at most CPS chunks."""
    out = []
    c = 0
    while c < nchunks:
        n = min(CPS, nchunks - c)
        out.append((c, n))
        c += n
    return out


def build_nc(nkc, nqc, split_waits=True):
    """nkc: kv chunks (128 rows each) per pair; nqc: q chunks per pair."""
    nc = bass.Bass()
    # Host-prepped layouts, all per-partition contiguous:
    #  k:  [pair][128, nkc*64]  chunk c cols [64c,64c+64) = rows 128c+p
    #  v:  [pair][128, nkc*65]  [v | 1] augmented
    #  qt: [duo][128, nqc*128]  partition 0-63 even pair dims, 64-127 odd;
    #                           col j = compacted q row j
    #  out:[pair][128, nqc*64]  chunk c cols = rows 128c+p (row layout)
    k_ext = nc.declare_dram_parameter("k", [PAIRS, 128, nkc * 64], BF16, isOutput=False)
    v_ext = nc.declare_dram_parameter("v", [PAIRS, 128, nkc * 65], BF16, isOutput=False)
    q_ext = nc.declare_dram_parameter("qt", [DUOS, 128, nqc * 128], BF16, isOutput=False)
    out_ext = nc.declare_dram_parameter(
        "out", [PAIRS, 128, nqc * 64], BF16, isOutput=True
    )

    A_max = mybir.AluOpType.max
    A_add = mybir.AluOpType.add
    A_min = mybir.AluOpType.min
    A_mult = mybir.AluOpType.mult
    EXP = mybir.ActivationFunctionType.Exp

    # Small first slab so the first MM1 issues early, then one big slab:
    # coarse ops amortize DVE/ACT fixed costs and per-op semaphore traffic.
    kslabs = [(0, 4), (4, nkc - 4)] if nkc > 4 else [(0, nkc)]
    if nqc > 21:
        qslabs = [(0, 7), (7, 14), (21, nqc - 21)]
    elif nqc > 7:
        qslabs = [(0, 7), (7, nqc - 7)]
    else:
        qslabs = [(0, nqc)]

    with tile.TileContext(nc, pool_alloc_mode="queue") as tc:
        from contextlib import ExitStack

        with ExitStack() as ctx:
            P = lambda name, bufs, space="SBUF": ctx.enter_context(
                tc.tile_pool(name=name, bufs=bufs, space=space)
            )
            k_pool = P("kslab", 2)
            v_pool = P("vslab", 2)
            e_pool = P("eslab", 2)
            r_pool = P("rslab", 2)
            ke_pool = P("keslab", 2)
            q_pool = P("qslab", 2)
            eq_pool = P("eqslab", 2)
            rq_pool = P("rqslab", 2)
            qe_pool = P("qeslab", 2)
            osb_pool = P("osb", 2)
            kv128_pool = P("kv128", 2)
            rec_pool = P("rec", 8)
            o_pool = P("oslab", 4)
            kv_ps_pool = P("kvps", 2, "PSUM")
            o_ps_pool = P("ops", 5, "PSUM")

            def phase_k_slab(d, c0, ncs, kv_ps):
                """One duo-merged slab (ncs chunks of both pairs): DMA both
                pairs into one tile, single elu chain, MM1 per pair into the
                two col groups of kv_ps."""
                w = ncs * 64
                ksl = k_pool.tile([128, 2 * w], BF16, tag="ksl")
                nc.sync.dma_start(ksl[:, 0:w], k_ext[2 * d][:, c0 * 64 : c0 * 64 + w])
                nc.sync.dma_start(
                    ksl[:, w : 2 * w], k_ext[2 * d + 1][:, c0 * 64 : c0 * 64 + w]
                )
                vsl = v_pool.tile([128, 2 * ncs * 65], BF16, tag="vsl")
                nc.sync.dma_start(
                    vsl[:, 0 : ncs * 65], v_ext[2 * d][:, c0 * 65 : (c0 + ncs) * 65]
                )
                nc.sync.dma_start(
                    vsl[:, ncs * 65 : 2 * ncs * 65],
                    v_ext[2 * d + 1][:, c0 * 65 : (c0 + ncs) * 65],
                )
                e = e_pool.tile([128, 2 * w], BF16, tag="e")
                nc.scalar.activation(e[:], ksl[:], EXP)
                r = r_pool.tile([128, 2 * w], BF16, tag="r")
                nc.vector.tensor_scalar(r[:], ksl[:], 0.0, 1.0, A_max, A_add)
                ke = ke_pool.tile([128, 2 * w], BF16, tag="ke")
                nc.vector.tensor_tensor(ke[:], e[:], r[:], A_min)
                v3 = vsl[:].rearrange("p (q c e) -> p q c e", q=2, e=65)
                for par in range(2):
                    half = par * 64
                    for c in range(ncs):
                        cc = c0 + c
                        nc.tensor.matmul(
                            kv_ps[half : half + 64, :],
                            ke[:, par * w + c * 64 : par * w + (c + 1) * 64],
                            v3[:, par, c, :],
                            start=(cc == 0),
                            stop=(cc == nkc - 1),
                        )

            def phase_kv_stack(kv_ps):
                """Copy the duo's stacked [128,65] KV PSUM to bf16 SBUF."""
                kv128 = kv128_pool.tile([128, 65], BF16, tag="kv128")
                nc.scalar.copy(kv128[:], kv_ps[:])
                return kv128

            def phase_q_prep(d, c0, ncs):
                """KV-independent q work: DMA + elu on the duo-packed q_t.
                Sub-ops of 7 chunks align with the epilogue groups."""
                qsl = q_pool.tile([128, ncs * 128], BF16, tag="qsl")
                nc.sync.dma_start(qsl[:], q_ext[d][:, c0 * 128 : (c0 + ncs) * 128])
                eq = eq_pool.tile([128, ncs * 128], BF16, tag="eq")
                nc.scalar.activation(eq[:], qsl[:], EXP)
                rq = rq_pool.tile([128, ncs * 128], BF16, tag="rq")
                nc.vector.tensor_scalar(rq[:], qsl[:], 0.0, 1.0, A_max, A_add)
                qe = qe_pool.tile([128, ncs * 128], BF16, tag="qe")
                nc.vector.tensor_tensor(qe[:], eq[:], rq[:], A_min)
                return qe

            def phase_q_mm(d, c0, ncs, kv128, qe, osl, on_gpsimd=False):
                """KV-dependent: MM2 chunks, epilogue, out DMA per half-slab.
                on_gpsimd: route this slab's broadcast-multiply through an ACT
                PSUM->SBUF copy + GpSimd multiply to offload the DVE."""
                for par in range(2):
                    half = par * 64
                    for g0 in range(0, ncs, 7):
                        gn = min(7, ncs - g0)
                        o_ps = o_ps_pool.tile([128, 455], F32, tag="ops")
                        for m in range(gn):
                            nc.tensor.matmul(
                                o_ps[:, m * 65 : (m + 1) * 65],
                                qe[half : half + 64, (g0 + m) * 128 : (g0 + m + 1) * 128],
                                kv128[half : half + 64, :],
                                start=True,
                                stop=True,
                            )
                        o3 = o_ps[:, 0 : gn * 65].rearrange("p (c e) -> p c e", e=65)
                        den = o3[:, :, 64:65].rearrange("p c e -> p (c e)")
                        rec = rec_pool.tile([128, gn], F32, tag="rec")
                        nc.vector.reciprocal(rec[:], den)
                        recb = _bcast_inner(rec[:], 64)
                        cols = slice((c0 + g0) * 64, (c0 + g0 + gn) * 64)
                        ov = osl[par][:, cols].rearrange("p (c e) -> p c e", e=64)
                        if on_gpsimd:
                            osb = osb_pool.tile([128, gn * 65], BF16, tag="osb")
                            nc.scalar.copy(osb[:], o_ps[:, 0 : gn * 65])
                            numsb = osb[:].rearrange("p (c e) -> p c e", e=65)[
                                :, :, 0:64
                            ]
                            nc.gpsimd.tensor_tensor(ov, numsb, recb, A_mult)
                        else:
                            nc.vector.tensor_tensor(ov, o3[:, :, 0:64], recb, A_mult)
                    # out DMA per (slab, parity), big slabs split in half;
                    # alternate Sync/Scalar HWDGE queues
                    halves = (
                        [(0, ncs)] if ncs <= 7 else [(0, ncs // 2), (ncs // 2, ncs)]
                    )
                    for hi, (h0, h1) in enumerate(halves):
                        cols = slice((c0 + h0) * 64, (c0 + h1) * 64)
                        eng = nc.sync if (par + hi) % 2 == 0 else nc.scalar
                        eng.dma_start(out_ext[2 * d + par][:, cols], osl[par][:, cols])

            # Software pipeline over duos: A(d) = k slabs + MM1 (KV build);
            # B(d) = per-slab q prep + MM2 + epilogue. B(d) overlaps A(d+1).
            def phase_A(d):
                # even pair accumulates into PSUM partitions 0-63 (array col
                # group 0), odd pair into 64-127 (col group 64) of one bank.
                kv_ps = kv_ps_pool.tile([128, 65], F32, tag="kvps")
                for (c0, ncs) in kslabs:
                    phase_k_slab(d, c0, ncs, kv_ps)
                return phase_kv_stack(kv_ps)

            def phase_B_slab(d, c0, ncs, kv128, osl):
                qe = phase_q_prep(d, c0, ncs)
                phase_q_mm(d, c0, ncs, kv128, qe, osl, on_gpsimd=(ncs <= 7))

            def new_osl(d):
                return [
                    o_pool.tile([128, nqc * 64], BF16, tag="osl", name=f"osl{d}_{par}")
                    for par in range(2)
                ]

            kv128 = phase_A(0)
            for d in range(DUOS):
                osl = new_osl(d)
                if d + 1 < DUOS:
                    # interleave this duo's B slabs with the next duo's K build
                    # (B first so its ACT/DVE work stays ahead in queue order)
                    kv_ps = kv_ps_pool.tile([128, 65], F32, tag="kvps")
                    for i in range(max(len(kslabs), len(qslabs))):
                        if i < len(qslabs):
                            phase_B_slab(d, qslabs[i][0], qslabs[i][1], kv128, osl)
                        if i < len(kslabs):
                            phase_k_slab(d + 1, kslabs[i][0], kslabs[i][1], kv_ps)
                    kv128 = phase_kv_stack(kv_ps)
                else:
                    for (c0, ncs) in qslabs:
                        phase_B_slab(d, c0, ncs, kv128, osl)
    if split_waits:
        _split_multi_waits(nc)
    return nc


_NC_CACHE = {}


def _get_nc(nkc, nqc):
    key = (nkc, nqc)
    if key not in _NC_CACHE:
        _NC_CACHE[key] = build_nc(nkc, nqc)
    return _NC_CACHE[key]


def _pad128(n):
    return max(128, (n + 127) // 128 * 128)


def kernel(q, k, v, q_mask, kv_mask):
    global LAST_RESULT
    q = np.asarray(q, dtype=np.float32)
    k = np.asarray(k, dtype=np.float32)
    v = np.asarray(v, dtype=np.float32)
    q_mask = np.asarray(q_mask).astype(bool)
    kv_mask = np.asarray(kv_mask).astype(bool)

    kv_idx = [np.nonzero(kv_mask[b])[0] for b in range(B)]
    q_idx = [np.nonzero(q_mask[b])[0] for b in range(B)]
    nkv = _pad128(max(len(ix) for ix in kv_idx))
    nq = _pad128(max(len(ix) for ix in q_idx))
    nkc, nqc = nkv // 128, nq // 128

    in_maps = []
    for core in range(8):
        b = core // 2
        h0 = 4 * (core % 2)
        ki, qi = kv_idx[b], q_idx[b]
        # k compacted: pad rows -> -300 so ke = elu+1 = 0 exactly (bf16)
        kc = np.full((PAIRS, nkv, D), -300.0, dtype=np.float32)
        kc[:, : len(ki)] = k[b, h0 : h0 + 4][:, ki]
        # v compacted and augmented with the ones column (ksum)
        vc = np.zeros((PAIRS, nkv, D + 1), dtype=np.float32)
        vc[:, : len(ki), :D] = v[b, h0 : h0 + 4][:, ki]
        vc[:, : len(ki), D] = 1.0
        # q compacted, transposed, duo-packed: [DUOS, 128, nq]
        qt = np.zeros((DUOS, 128, nq), dtype=np.float32)
        for d in range(DUOS):
            qt[d, 0:64, : len(qi)] = q[b, h0 + 2 * d][qi].T
            qt[d, 64:128, : len(qi)] = q[b, h0 + 2 * d + 1][qi].T
        in_maps.append(
            {
                # chunk-major layouts: [128 part, chunks*width], partition p
                # of chunk c = compacted row 128c+p
                "k": np.ascontiguousarray(
                    kc.reshape(PAIRS, nkc, 128, D).transpose(0, 2, 1, 3)
                ).reshape(PAIRS, 128, nkc * D).astype(ml_dtypes.bfloat16),
                "v": np.ascontiguousarray(
                    vc.reshape(PAIRS, nkc, 128, D + 1).transpose(0, 2, 1, 3)
                ).reshape(PAIRS, 128, nkc * (D + 1)).astype(ml_dtypes.bfloat16),
                "qt": qt.astype(ml_dtypes.bfloat16),
            }
        )

    nc = _get_nc(nkc, nqc)
    res = run_bass_kernel_spmd(
        nc,
        in_maps,
        core_ids=list(range(8)),
        trace=os.environ.get("KERNEL_TRACE", "0") == "1",
    )
    LAST_RESULT = res

    out = np.zeros((B, H, S, D), dtype=np.float32)
    for core in range(8):
        b = core // 2
        h0 = 4 * (core % 2)
        qi = q_idx[b]
        oc = res.results[core]["out"].astype(np.float32)  # [PAIRS, 128, nqc*64]
        oc = oc.reshape(PAIRS, 128, nqc, D).transpose(0, 2, 1, 3).reshape(
            PAIRS, nq, D
        )
        out[b, h0 : h0 + 4][:, qi] = oc[:, : len(qi)]
    return out


# revision 39
# speedup vs baseline: 1.1850x; 1.1850x over previous
"""Linear-attention (elu feature map) Bass kernel for Trainium2, 8 NeuronCores.

Problem: B=4, H=8, S=8192, D=64 fp32.
  qe = elu(q)+1, ke = elu(k)+1, masked by q_mask/kv_mask
  KV = ke^T @ ve (contract S), ksum = sum_s ke*km
  out = (qe @ KV) / (qe . ksum + 1e-6) * q_mask

Sharding: (B,H) = 32 pairs over 8 cores -> 4 pairs/core, one b per core.

v2 design (host does layout only; all FLOPs on device):
  * Row compaction: kv_mask/q_mask zero ~half the rows and every
    contraction is order-invariant over S, so the host gathers only the
    unmasked rows (padded to a common 128-multiple across cores for
    SPMD). Masked k rows never reach the device; masked q rows are
    zero-filled on the host during scatter-back. This halves DMA and
    every engine's work and removes all on-device mask multiplies.
  * k is shipped compacted; padding rows get -300 so ke = elu+1 = 0.
  * v is shipped as [v | 1] (65 wide); the ones column yields ksum in
    the same accumulated matmul that makes KV.
  * q is shipped pre-transposed and duo-packed: q_t[duo] = [128, NQ]
    bf16, partitions 0-63 = even pair's 64 dims, 64-127 = odd pair's.
    MM2 weights slice straight out of this tile - no PE transposes.
  * elu+1 == min(exp(x), relu(x)+1) exactly: exp on ACT,
    relu+1 (dual-op tensor_scalar, 4x) and min (2x) on DVE, all bf16.
  * MM1 per pair: 128-row chunks accumulate ke^T @ [v|1]; the even
    pair lands in PSUM partitions 0-63 (array col group 0), the odd
    pair in 64-127 (col group 64, via out.base_partition), so one
    [128, 65] bank holds the duo's stacked KV for MM2's two halves.
  * MM2 per 128-col chunk of q_t: out[128s, 65] = qe_chunk^T.T @ kv128
    half; groups of 7 same-parity chunks share a PSUM bank; epilogue
    per bank: rec = recip(den cols), out = num * rec (broadcast AP) on
    DVE, except the small-slab groups which detour through an ACT
    PSUM->SBUF copy + GpSimd multiply to offload the DVE.
  * Software pipeline at duo granularity: A(d) = k/v slabs + MM1;
    B(d) = q prep + MM2 + epilogue + out DMA; B(d) overlaps A(d+1).
    Slabs are (4, rest): a small first slab starts the MM stream early,
    the big slab amortizes DVE/ACT fixed costs and semaphore traffic.
"""
import os
import sys

sys.path.insert(0, "/opt/trn_rl_repo")

import numpy as np
import ml_dtypes

import concourse.bass as bass
import concourse.tile as tile
from concourse import mybir
import bass_rust
from concourse.bass_utils import run_bass_kernel_spmd

B, H, S, D = 4, 8, 8192, 64
PAIRS = 4
DUOS = 2
CPS = 16  # chunks per slab (slab = CPS*128 rows)
F32 = mybir.dt.float32
BF16 = mybir.dt.bfloat16

LAST_RESULT = None


def _split_multi_waits(nc, max_waits=1):
    """walrus setupSyncWait rejects >1 sem wait on one instruction; hoist
    extras onto preceding NoOps on the same engine."""
    for fn in nc.m.functions:
        for bb in fn.blocks:
            insts = list(bb.instructions)
            out = []
            changed = False
            for inst in insts:
                si = getattr(inst, "sync_info", None)
                ow = list(si.on_wait) if si is not None and si.on_wait else []
                if len(ow) > max_waits:
                    changed = True
                    for j, w in enumerate(ow[:-max_waits]):
                        nop = mybir.InstNoOp(
                            name=f"{inst.name}-splitw{j}", ins=[], outs=[]
                        )
                        nop.engine = inst.engine
                        nop.sync_info = bass_rust.SyncInfo(on_wait=[w], on_update=[])
                        out.append(nop)
                    inst.sync_info = bass_rust.SyncInfo(
                        on_wait=ow[-max_waits:], on_update=list(si.on_update or [])
                    )
                out.append(inst)
            if changed:
                bb.instructions = out


def _bcast_inner(ap, n):
    """Append a step-0 inner dim reading each element n times."""
    ap = ap[:, :]
    ap.ap.append([0, n])
    return ap


def _slabs(nchunks):
    """Split nchunks into slabs of at most CPS chunks."""
    out = []
    c = 0
    while c < nchunks:
        n = min(CPS, nchunks - c)
        out.append((c, n))
        c += n
    return out


def build_nc(nkc, nqc, split_waits=True):
    """nkc: kv chunks (128 rows each) per pair; nqc: q chunks per pair."""
    nc = bass.Bass()
    # Host-prepped layouts, all per-partition contiguous:
    #  k:  [pair][128, nkc*64]  chunk c cols [64c,64c+64) = rows 128c+p
    #  v:  [pair][128, nkc*65]  [v | 1] augmented
    #  qt: [duo][128, nqc*128]  partition 0-63 even pair dims, 64-127 odd;
    #                           col j = compacted q row j
    #  out:[pair][128, nqc*64]  chunk c cols = rows 128c+p (row layout)
    k_ext = nc.declare_dram_parameter("k", [PAIRS, 128, nkc * 64], BF16, isOutput=False)
    v_ext = nc.declare_dram_parameter("v", [PAIRS, 128, nkc * 65], BF16, isOutput=False)
    q_ext = nc.declare_dram_parameter("qt", [DUOS, 128, nqc * 128], BF16, isOutput=False)
    out_ext = nc.declare_dram_parameter(
        "out", [PAIRS, 128, nqc * 64], BF16, isOutput=True
    )

    A_max = mybir.AluOpType.max
    A_add = mybir.AluOpType.add
    A_min = mybir.AluOpType.min
    A_mult = mybir.AluOpType.mult
    EXP = mybir.ActivationFunctionType.Exp

    # Small first slab so the first MM1 issues early, then one big slab:
    # coarse ops amortize DVE/ACT fixed costs and per-op semaphore traffic.
    kslabs = [(0, 4), (4, nkc - 4)] if nkc > 4 else [(0, nkc)]
    qslabs = [(0, 7), (7, nqc - 7)] if nqc > 7 else [(0, nqc)]

    with tile.TileContext(nc, pool_alloc_mode="queue") as tc:
        from contextlib import ExitStack

        with ExitStack() as ctx:
            P = lambda name, bufs, space="SBUF": ctx.enter_context(
                tc.tile_pool(name=name, bufs=bufs, space=space)
            )
            k_pool = P("kslab", 2)
            v_pool = P("vslab", 2)
            e_pool = P("eslab", 2)
            r_pool = P("rslab", 2)
            ke_pool = P("keslab", 2)
            q_pool = P("qslab", 2)
            eq_pool = P("eqslab", 2)
            rq_pool = P("rqslab", 2)
            qe_pool = P("qeslab", 2)
            osb_pool = P("osb", 2)
            kv128_pool = P("kv128", 2)
            rec_pool = P("rec", 8)
            o_pool = P("oslab", 4)
            kv_ps_pool = P("kvps", 2, "PSUM")
            o_ps_pool = P("ops", 5, "PSUM")

            def phase_k_slab(d, c0, ncs, kv_ps):
                """One duo-merged slab (ncs chunks of both pairs): DMA both
                pairs into one tile, single elu chain, MM1 per pair into the
                two col groups of kv_ps."""
                w = ncs * 64
                ksl = k_pool.tile([128, 2 * w], BF16, tag="ksl")
                nc.sync.dma_start(ksl[:, 0:w], k_ext[2 * d][:, c0 * 64 : c0 * 64 + w])
                nc.sync.dma_start(
                    ksl[:, w : 2 * w], k_ext[2 * d + 1][:, c0 * 64 : c0 * 64 + w]
                )
                vsl = v_pool.tile([128, 2 * ncs * 65], BF16, tag="vsl")
                nc.sync.dma_start(
                    vsl[:, 0 : ncs * 65], v_ext[2 * d][:, c0 * 65 : (c0 + ncs) * 65]
                )
                nc.sync.dma_start(
                    vsl[:, ncs * 65 : 2 * ncs * 65],
                    v_ext[2 * d + 1][:, c0 * 65 : (c0 + ncs) * 65],
                )
                e = e_pool.tile([128, 2 * w], BF16, tag="e")
                nc.scalar.activation(e[:], ksl[:], EXP)
                r = r_pool.tile([128, 2 * w], BF16, tag="r")
                nc.vector.tensor_scalar(r[:], ksl[:], 0.0, 1.0, A_max, A_add)
                ke = ke_pool.tile([128, 2 * w], BF16, tag="ke")
                nc.vector.tensor_tensor(ke[:], e[:], r[:], A_min)
                v3 = vsl[:].rearrange("p (q c e) -> p q c e", q=2, e=65)
                for par in range(2):
                    half = par * 64
                    for c in range(ncs):
                        cc = c0 + c
                        nc.tensor.matmul(
                            kv_ps[half : half + 64, :],
                            ke[:, par * w + c * 64 : par * w + (c + 1) * 64],
                            v3[:, par, c, :],
                            start=(cc == 0),
                            stop=(cc == nkc - 1),
                        )

            def phase_kv_stack(kv_ps):
                """Copy the duo's stacked [128,65] KV PSUM to bf16 SBUF."""
                kv128 = kv128_pool.tile([128, 65], BF16, tag="kv128")
                nc.scalar.copy(kv128[:], kv_ps[:])
                return kv128

            def phase_q_prep(d, c0, ncs):
                """KV-independent q work: DMA + elu on the duo-packed q_t.
                Sub-ops of 7 chunks align with the epilogue groups."""
                qsl = q_pool.tile([128, ncs * 128], BF16, tag="qsl")
                nc.sync.dma_start(qsl[:], q_ext[d][:, c0 * 128 : (c0 + ncs) * 128])
                eq = eq_pool.tile([128, ncs * 128], BF16, tag="eq")
                nc.scalar.activation(eq[:], qsl[:], EXP)
                rq = rq_pool.tile([128, ncs * 128], BF16, tag="rq")
                nc.vector.tensor_scalar(rq[:], qsl[:], 0.0, 1.0, A_max, A_add)
                qe = qe_pool.tile([128, ncs * 128], BF16, tag="qe")
                nc.vector.tensor_tensor(qe[:], eq[:], rq[:], A_min)
                return qe

            def phase_q_mm(d, c0, ncs, kv128, qe, osl, on_gpsimd=False):
                """KV-dependent: MM2 chunks, epilogue, out DMA per half-slab.
                on_gpsimd: route this slab's broadcast-multiply through an ACT
                PSUM->SBUF copy + GpSimd multiply to offload the DVE."""
                for par in range(2):
                    half = par * 64
                    for g0 in range(0, ncs, 7):
                        gn = min(7, ncs - g0)
                        o_ps = o_ps_pool.tile([128, 455], F32, tag="ops")
                        for m in range(gn):
                            nc.tensor.matmul(
                                o_ps[:, m * 65 : (m + 1) * 65],
                                qe[half : half + 64, (g0 + m) * 128 : (g0 + m + 1) * 128],
                                kv128[half : half + 64, :],
                                start=True,
                                stop=True,
                            )
                        o3 = o_ps[:, 0 : gn * 65].rearrange("p (c e) -> p c e", e=65)
                        den = o3[:, :, 64:65].rearrange("p c e -> p (c e)")
                        rec = rec_pool.tile([128, gn], F32, tag="rec")
                        nc.vector.reciprocal(rec[:], den)
                        recb = _bcast_inner(rec[:], 64)
                        cols = slice((c0 + g0) * 64, (c0 + g0 + gn) * 64)
                        ov = osl[par][:, cols].rearrange("p (c e) -> p c e", e=64)
                        if on_gpsimd:
                            osb = osb_pool.tile([128, gn * 65], BF16, tag="osb")
                            nc.scalar.copy(osb[:], o_ps[:, 0 : gn * 65])
                            numsb = osb[:].rearrange("p (c e) -> p c e", e=65)[
                                :, :, 0:64
                            ]
                            nc.gpsimd.tensor_tensor(ov, numsb, recb, A_mult)
                        else:
                            nc.vector.tensor_tensor(ov, o3[:, :, 0:64], recb, A_mult)
                    # out DMA per (slab, parity), big slabs split in half;
                    # alternate Sync/Scalar HWDGE queues
                    halves = (
                        [(0, ncs)] if ncs <= 7 else [(0, ncs // 2), (ncs // 2, ncs)]
                    )
                    for hi, (h0, h1) in enumerate(halves):
                        cols = slice((c0 + h0) * 64, (c0 + h1) * 64)
                        eng = nc.sync if (par + hi) % 2 == 0 else nc.scalar
                        eng.dma_start(out_ext[2 * d + par][:, cols], osl[par][:, cols])

            # Software pipeline over duos: A(d) = k slabs + MM1 (KV build);
            # B(d) = per-slab q prep + MM2 + epilogue. B(d) overlaps A(d+1).
            def phase_A(d):
                # even pair accumulates into PSUM partitions 0-63 (array col
                # group 0), odd pair into 64-127 (col group 64) of one bank.
                kv_ps = kv_ps_pool.tile([128, 65], F32, tag="kvps")
                for (c0, ncs) in kslabs:
                    phase_k_slab(d, c0, ncs, kv_ps)
                return phase_kv_stack(kv_ps)

            def phase_B_slab(d, c0, ncs, kv128, osl):
                qe = phase_q_prep(d, c0, ncs)
                phase_q_mm(d, c0, ncs, kv128, qe, osl, on_gpsimd=(ncs <= 7))

            def new_osl(d):
                return [
                    o_pool.tile([128, nqc * 64], BF16, tag="osl", name=f"osl{d}_{par}")
                    for par in range(2)
                ]

            kv128 = phase_A(0)
            for d in range(DUOS):
                osl = new_osl(d)
                if d + 1 < DUOS:
                    # interleave this duo's B slabs with the next duo's K build
                    # (B first so its ACT/DVE work stays ahead in queue order)
                    kv_ps = kv_ps_pool.tile([128, 65], F32, tag="kvps")
                    for i in range(max(len(kslabs), len(qslabs))):
                        if i < len(qslabs):
                            phase_B_slab(d, qslabs[i][0], qslabs[i][1], kv128, osl)
                        if i < len(kslabs):
                            phase_k_slab(d + 1, kslabs[i][0], kslabs[i][1], kv_ps)
                    kv128 = phase_kv_stack(kv_ps)
                else:
                    for (c0, ncs) in qslabs:
                        phase_B_slab(d, c0, ncs, kv128, osl)
    if split_waits:
        _split_multi_waits(nc)
    return nc


_NC_CACHE = {}


def _get_nc(nkc, nqc):
    key = (nkc, nqc)
    if key not in _NC_CACHE:
        _NC_CACHE[key] = build_nc(nkc, nqc)
    return _NC_CACHE[key]


def _pad128(n):
    return max(128, (n + 127) // 128 * 128)


def kernel(q, k, v, q_mask, kv_mask):
    global LAST_RESULT
    q = np.asarray(q, dtype=np.float32)
    k = np.asarray(k, dtype=np.float32)
    v = np.asarray(v, dtype=np.float32)
    q_mask = np.asarray(q_mask).astype(bool)
    kv_mask = np.asarray(kv_mask).astype(bool)

    kv_idx = [np.nonzero(kv_mask[b])[0] for b in range(B)]
    q_idx = [np.nonzero(q_mask[b])[0] for b in range(B)]
    nkv = _pad128(max(len(ix) for ix in kv_idx))
    nq = _pad128(max(len(ix) for ix in q_idx))
    nkc, nqc = nkv // 128, nq // 128

    in_maps = []
    for core in range(8):
        b = core // 2
        h0 = 4 * (core % 2)
        ki, qi = kv_idx[b], q_idx[b]
        # k compacted: pad rows -> -300 so ke = elu+1 = 0 exactly (bf16)
        kc = np.full((PAIRS, nkv, D), -300.0, dtype=np.float32)
        kc[:, : len(ki)] = k[b, h0 : h0 + 4][:, ki]
        # v compacted and augmented with the ones column (ksum)
        vc = np.zeros((PAIRS, nkv, D + 1), dtype=np.float32)
        vc[:, : len(ki), :D] = v[b, h0 : h0 + 4][:, ki]
        vc[:, : len(ki), D] = 1.0
        # q compacted, transposed, duo-packed: [DUOS, 128, nq]
        qt = np.zeros((DUOS, 128, nq), dtype=np.float32)
        for d in range(DUOS):
            qt[d, 0:64, : len(qi)] = q[b, h0 + 2 * d][qi].T
            qt[d, 64:128, : len(qi)] = q[b, h0 + 2 * d + 1][qi].T
        in_maps.append(
            {
                # chunk-major layouts: [128 part, chunks*width], partition p
                # of chunk c = compacted row 128c+p
                "k": np.ascontiguousarray(
                    kc.reshape(PAIRS, nkc, 128, D).transpose(0, 2, 1, 3)
                ).reshape(PAIRS, 128, nkc * D).astype(ml_dtypes.bfloat16),
                "v": np.ascontiguousarray(
                    vc.reshape(PAIRS, nkc, 128, D + 1).transpose(0, 2, 1, 3)
                ).reshape(PAIRS, 128, nkc * (D + 1)).astype(ml_dtypes.bfloat16),
                "qt": qt.astype(ml_dtypes.bfloat16),
            }
        )

    nc = _get_nc(nkc, nqc)
    res = run_bass_kernel_spmd(
        nc,
        in_maps,
        core_ids=list(range(8)),
        trace=os.environ.get("KERNEL_TRACE", "0") == "1",
    )
    LAST_RESULT = res

    out = np.zeros((B, H, S, D), dtype=np.float32)
    for core in range(8):
        b = core // 2
        h0 = 4 * (core % 2)
        qi = q_idx[b]
        oc = res.results[core]["out"].astype(np.float32)  # [PAIRS, 128, nqc*64]
        oc = oc.reshape(PAIRS, 128, nqc, D).transpose(0, 2, 1, 3).reshape(
            PAIRS, nq, D
        )
        out[b, h0 : h0 + 4][:, qi] = oc[:, : len(qi)]
    return out
